# revision 4
# baseline (speedup 1.0000x reference)
"""MoE routed dense layer (nn_MultiHeadDense): y[b] = x[b] @ W[idx[b]] + bias[idx[b]].

Full shapes: inputs [4096,1024] f32, indices [4096] int, kernel [8,1024,1024] f32,
bias [8,1024] f32 -> out [4096,1024] f32.

Sharding strategy (expert-parallel, H == n_cores == 8): core h owns expert h's
weight [1024,1024] and processes exactly the rows routed to expert h. The host
computes the per-expert row lists from `indices`, gathers each expert's rows
into a zero-padded transposed activation block XT_h [D, C] (C = padded max
group size), and scatters the per-core outputs back into the full [B, F]
result. This does 1/8th the FLOPs of the dense all-heads reference and loads
each expert weight exactly once, on exactly one core.

On-device per core: Y[c, f] = sum_k XT[k*128:(k+1)*128, c].T @ W[k*128:.., f]
accumulated in PSUM over the 8 k-tiles, bias added during the PSUM->SBUF
eviction. Matmuls run as float32r (full PE rate at N=512) on f32 data.
"""

from contextlib import ExitStack

import numpy as np

import concourse.bass as bass
import concourse.tile as tile
from concourse import bacc, mybir
from concourse.bass_utils import run_bass_kernel_spmd

F32 = mybir.dt.float32
F32R = mybir.dt.float32r

P = 128          # SBUF partitions / matmul tile edge
NTILE = 512      # matmul moving free dim (one fp32 PSUM bank)


def _build(nc: bass.Bass, C: int, D: int, F: int):
    xt = nc.dram_tensor("xt", (D, C), F32R, kind="ExternalInput").ap()
    w = nc.dram_tensor("w", (D, F), F32R, kind="ExternalInput").ap()
    b = nc.dram_tensor("b", (P, F), F32, kind="ExternalInput").ap()
    y = nc.dram_tensor("y", (C, F), F32, kind="ExternalOutput").ap()

    KT = D // P       # k-tiles (contraction)
    MT = C // P       # m-tiles (rows of this expert's batch)
    NT = F // NTILE   # n-tiles (output features)

    with tile.TileContext(nc) as tc, ExitStack() as ctx:
        wp = ctx.enter_context(tc.tile_pool(name="wp", bufs=KT))
        xp = ctx.enter_context(tc.tile_pool(name="xp", bufs=KT))
        bp = ctx.enter_context(tc.tile_pool(name="bp", bufs=1))
        pp = ctx.enter_context(tc.tile_pool(name="pp", bufs=2 * NT, space="PSUM"))
        yp = ctx.enter_context(tc.tile_pool(name="yp", bufs=3))

        bias_t = bp.tile([P, F], F32)
        nc.sync.dma_start(bias_t[:], b[:])

        w_tiles = []
        x_tiles = []
        for k in range(KT):
            wt = wp.tile([P, F], F32R, name=f"w{k}", tag="w")
            nc.sync.dma_start(wt[:], w[k * P:(k + 1) * P, :])
            w_tiles.append(wt)
            xtt = xp.tile([P, C], F32R, name=f"x{k}", tag="x")
            nc.sync.dma_start(xtt[:], xt[k * P:(k + 1) * P, :])
            x_tiles.append(xtt)

        for m in range(MT):
            ps = []
            for n in range(NT):
                pst = pp.tile([P, NTILE], F32, name=f"ps{m}_{n}", tag="ps")
                ps.append(pst)
            for k in range(KT):
                lhs = x_tiles[k][:, m * P:(m + 1) * P]
                for n in range(NT):
                    nc.tensor.matmul(
                        ps[n][:],
                        lhsT=lhs,
                        rhs=w_tiles[k][:, n * NTILE:(n + 1) * NTILE],
                        start=(k == 0),
                        stop=(k == KT - 1),
                    )
            yt = yp.tile([P, F], F32, name=f"y{m}", tag="y")
            for n in range(NT):
                nc.vector.tensor_add(
                    yt[:, n * NTILE:(n + 1) * NTILE],
                    ps[n][:],
                    bias_t[:, n * NTILE:(n + 1) * NTILE],
                )
            nc.sync.dma_start(y[m * P:(m + 1) * P, :], yt[:])


LAST_PROFILE = {}


def kernel(inputs, indices, kernel, bias, _trace=False):
    x = np.ascontiguousarray(np.asarray(inputs), dtype=np.float32)
    idx = np.asarray(indices).astype(np.int64)
    wk = np.asarray(kernel, dtype=np.float32)
    bv = np.asarray(bias, dtype=np.float32)

    B, D = x.shape
    H, _, F = wk.shape

    rows = [np.nonzero(idx == h)[0] for h in range(H)]
    maxc = max(len(r) for r in rows)
    C = max(((maxc + P - 1) // P) * P, P)

    in_maps = []
    for h in range(H):
        r = rows[h]
        xt = np.zeros((D, C), dtype=np.float32)
        xt[:, :len(r)] = x[r].T
        in_maps.append({
            "xt": xt,
            "w": np.ascontiguousarray(wk[h]),
            "b": np.broadcast_to(bv[h], (P, F)).copy(),
        })

    nc = bacc.Bacc(
        "TRN2", target_bir_lowering=False, debug=False, num_devices=H
    )
    _build(nc, C, D, F)
    nc.compile()

    trace_kwargs = (
        {"trace": True, "trace_cores": list(range(H)), "stitch_traces": False}
        if _trace
        else {}
    )
    res = run_bass_kernel_spmd(nc, in_maps, core_ids=list(range(H)), **trace_kwargs)
    if _trace:
        LAST_PROFILE.clear()
        LAST_PROFILE.update(
            exec_time_ns=res.exec_time_ns,
            mean_exec_time_ns=res.mean_exec_time_ns,
            max_exec_time_core_id=res.max_exec_time_core_id,
            trace=res.instructions_and_trace[1] if res.instructions_and_trace else None,
            profile_json=res.profile_json,
        )

    out = np.empty((B, F), dtype=np.float32)
    for h in range(H):
        r = rows[h]
        out[r] = res.results[h]["y"][:len(r)]
    return out
